# revision 4
# baseline (speedup 1.0000x reference)
"""KDE on a 20^3 grid, distributed across 8 TRN2 NeuronCores.

Math: kde[a] = sum_b K[a,b] * p[b], K[a,b] = coef * exp(-0.5 * d2[a,b]),
d2[a,b] = (x_a - x_b)^T A (x_a - x_b), then output = kde / sum(kde).
(coef cancels in the normalization, so it is never computed.)

Device algorithm (per core, rows sharded 8 ways -> 1000 rows/core):
  tile layout: partitions = b (kernel-source points, 63 chunks of 128 padded
  to 8064), free dim = i (this core's rows, 2 blocks of 500).
  d2[b,i] = q_b + q_i - 2*GA_i . x_b   with GA = x @ A, q = rowsum(x*GA)
  - one k=8 fp16 matmul per tile computes q_i - 2*GA_i.x_b
    (fp16 hi/lo split of -2*GA and q gives ~22-bit effective mantissa;
     the centered coords +-9.5, +-8.5, ... are exact in fp16)
  - ScalarE: E = exp(-0.5*in + bias_b), bias_b = -0.5*q_b  (per-partition)
  - accumulating matmul with p_b as stationary weights -> kde block [1, 500]
  - AllReduce the scalar normalizer, multiply by reciprocal, DMA out.
"""

import numpy as np

GRID = (20, 20, 20)
N = 8000
NCORES = 8
ROWS = N // NCORES          # 1000 rows per core
NCH = 63                    # b chunks of 128
NPAD = NCH * 128            # 8064
NBLK = 2
BLK = ROWS // NBLK          # 500

_PROGRAM = None


def _build_program():
    from contextlib import ExitStack

    import concourse.bacc as bacc
    import concourse.mybir as mybir
    import concourse.tile as tile

    f32 = mybir.dt.float32
    f16 = mybir.dt.float16

    nc = bacc.Bacc(
        "TRN2",
        target_bir_lowering=False,
        debug=False,
        num_devices=NCORES,
    )

    lc_d = nc.dram_tensor("lc", [8, NPAD], f16, kind="ExternalInput").ap()
    csq_d = nc.dram_tensor("csq", [9, NPAD], f32, kind="ExternalInput").ap()
    cov9_d = nc.dram_tensor("cov9", [9, 1], f32, kind="ExternalInput").ap()
    cov3_d = nc.dram_tensor("cov3", [3, 3], f32, kind="ExternalInput").ap()
    ctl_d = nc.dram_tensor("ctl", [3, ROWS], f32, kind="ExternalInput").ap()
    csql_d = nc.dram_tensor("csql", [9, ROWS], f32, kind="ExternalInput").ap()
    pcol_d = nc.dram_tensor("pcol", [128, NCH], f32, kind="ExternalInput").ap()
    out_d = nc.dram_tensor("out", [1, ROWS], f32, kind="ExternalOutput").ap()

    with tile.TileContext(nc) as tc, ExitStack() as ctx:
        const = ctx.enter_context(tc.tile_pool(name="const", bufs=1))
        work = ctx.enter_context(tc.tile_pool(name="work", bufs=3))
        psum_dp = ctx.enter_context(tc.tile_pool(name="psum_dp", bufs=3, space="PSUM"))
        psum_kp = ctx.enter_context(tc.tile_pool(name="psum_kp", bufs=2, space="PSUM"))
        psum_pre = ctx.enter_context(tc.tile_pool(name="psum_pre", bufs=1, space="PSUM"))
        dram = ctx.enter_context(tc.tile_pool(name="dram", bufs=1, space="DRAM"))

        # ---- input loads ----
        lc_sb = const.tile([8, NPAD], f16)
        nc.sync.dma_start(out=lc_sb[:], in_=lc_d[:])
        csq_sb = const.tile([9, NPAD], f32)
        nc.sync.dma_start(out=csq_sb[:], in_=csq_d[:])
        cov9 = const.tile([9, 1], f32)
        nc.sync.dma_start(out=cov9[:], in_=cov9_d[:])
        cov3 = const.tile([3, 3], f32)
        nc.sync.dma_start(out=cov3[:], in_=cov3_d[:])
        ctl_sb = const.tile([3, ROWS], f32)
        nc.sync.dma_start(out=ctl_sb[:], in_=ctl_d[:])
        csql_sb = const.tile([9, ROWS], f32)
        nc.sync.dma_start(out=csql_sb[:], in_=csql_d[:])
        pcol32 = const.tile([128, NCH], f32)
        nc.sync.dma_start(out=pcol32[:], in_=pcol_d[:])
        pcol16 = const.tile([128, NCH], f16)
        nc.vector.tensor_copy(pcol16[:], pcol32[:])

        # ---- bias column: qb[lane, chunk] = q of source point b, times -0.5 ----
        qp = psum_pre.tile([128, NCH], f32)
        for c in range(NCH):
            nc.tensor.matmul(
                qp[:, c : c + 1],
                lhsT=csq_sb[:, c * 128 : (c + 1) * 128],
                rhs=cov9[:],
                start=True,
                stop=True,
            )
        qbias = const.tile([128, NCH], f32)
        nc.vector.tensor_scalar_mul(qbias[:], qp[:], -0.5)

        # ---- local row factors: g2 = -2*GA_loc^T [3,ROWS], qloc [1,ROWS] ----
        g2 = const.tile([3, ROWS], f32)
        qloc = const.tile([1, ROWS], f32)
        for blk in range(NBLK):
            sl = slice(blk * BLK, (blk + 1) * BLK)
            gp = psum_pre.tile([3, BLK], f32)
            nc.tensor.matmul(
                gp[:], lhsT=cov3[:], rhs=ctl_sb[:, sl], start=True, stop=True
            )
            nc.vector.tensor_scalar_mul(g2[:, sl], gp[:], -2.0)
            qlp = psum_pre.tile([1, BLK], f32)
            nc.tensor.matmul(
                qlp[:], lhsT=cov9[:], rhs=csql_sb[:, sl], start=True, stop=True
            )
            nc.vector.tensor_copy(qloc[0:1, sl], qlp[:])

        # fp16 hi/lo split (compute engines need partition base 0; assemble
        # the 8-row factor tile with DMAs, which can write any partition)
        ghi = const.tile([3, ROWS], f16)
        nc.vector.tensor_copy(ghi[:], g2[:])
        ghi32 = const.tile([3, ROWS], f32)
        nc.vector.tensor_copy(ghi32[:], ghi[:])
        glo = const.tile([3, ROWS], f16)
        nc.vector.tensor_sub(glo[:], g2[:], ghi32[:])
        qhi = const.tile([1, ROWS], f16)
        nc.vector.tensor_copy(qhi[:], qloc[:])
        qhi32 = const.tile([1, ROWS], f32)
        nc.vector.tensor_copy(qhi32[:], qhi[:])
        qlo = const.tile([1, ROWS], f16)
        nc.vector.tensor_sub(qlo[:], qloc[:], qhi32[:])

        rfac = const.tile([8, ROWS], f16)
        nc.sync.dma_start(out=rfac[0:3, :], in_=ghi[:])
        nc.sync.dma_start(out=rfac[3:4, :], in_=qhi[:])
        nc.sync.dma_start(out=rfac[4:7, :], in_=glo[:])
        nc.sync.dma_start(out=rfac[7:8, :], in_=qlo[:])

        # ---- main loop ----
        kde_sb = const.tile([1, ROWS], f32)
        for blk in range(NBLK):
            sl = slice(blk * BLK, (blk + 1) * BLK)
            kp = psum_kp.tile([1, BLK], f32)
            for c in range(NCH):
                dp = psum_dp.tile([128, BLK], f32)
                nc.tensor.matmul(
                    dp[:],
                    lhsT=lc_sb[:, c * 128 : (c + 1) * 128],
                    rhs=rfac[:, sl],
                    start=True,
                    stop=True,
                )
                ek = work.tile([128, BLK], f16, tag="ek")
                nc.scalar.activation(
                    ek[:],
                    dp[:],
                    mybir.ActivationFunctionType.Exp,
                    bias=qbias[:, c : c + 1],
                    scale=-0.5,
                )
                nc.tensor.matmul(
                    kp[:],
                    lhsT=pcol16[:, c : c + 1],
                    rhs=ek[:],
                    start=(c == 0),
                    stop=(c == NCH - 1),
                )
            nc.vector.tensor_copy(kde_sb[0:1, sl], kp[:])

        # ---- normalizer: allreduce the local sum, scale, store ----
        ssum = const.tile([1, 1], f32)
        nc.vector.tensor_reduce(
            ssum[:], kde_sb[:], axis=mybir.AxisListType.X, op=mybir.AluOpType.add
        )
        ccin = dram.tile([1, 1], f32)
        ccout = dram.tile([1, 1], f32)
        nc.sync.dma_start(out=ccin[:], in_=ssum[:])
        nc.gpsimd.collective_compute(
            "AllReduce",
            mybir.AluOpType.add,
            replica_groups=[list(range(NCORES))],
            ins=[ccin.opt()],
            outs=[ccout.opt()],
        )
        stot = const.tile([1, 1], f32)
        nc.sync.dma_start(out=stot[:], in_=ccout[:])
        rec = const.tile([1, 1], f32)
        nc.vector.reciprocal(rec[:], stot[:])
        kout = const.tile([1, ROWS], f32)
        nc.vector.tensor_scalar_mul(kout[:], kde_sb[:], rec[:])
        nc.sync.dma_start(out=out_d[:], in_=kout[:])

    nc.compile()
    return nc


def _get_program():
    global _PROGRAM
    if _PROGRAM is None:
        _PROGRAM = _build_program()
    return _PROGRAM


def _host_inputs(space_probs, cov_inv):
    """Build the per-core input maps (host-side layout/shard prep only)."""
    p = np.asarray(space_probs, dtype=np.float32).reshape(-1)
    a = np.asarray(cov_inv, dtype=np.float32)

    idx = np.indices(GRID, dtype=np.float32).reshape(3, N)  # [3, N], i fastest-major
    c = idx - 9.5  # centered; values +-0.5..+-9.5 are exact in fp16

    lc = np.zeros((8, NPAD), dtype=np.float16)
    lc[0:3, :N] = c
    lc[3, :N] = 1.0
    lc[4:7, :N] = c
    lc[7, :N] = 1.0

    csq = np.zeros((9, NPAD), dtype=np.float32)
    k = 0
    for ai in range(3):
        for bi in range(3):
            csq[k, :N] = c[ai] * c[bi]
            k += 1

    pcol = np.zeros((NCH, 128), dtype=np.float32)
    pcol.reshape(-1)[:N] = p
    pcol = np.ascontiguousarray(pcol.T)  # [128, NCH]

    cov9 = np.ascontiguousarray(a.reshape(9, 1))
    cov3 = np.ascontiguousarray(a)

    in_maps = []
    for r in range(NCORES):
        sl = slice(r * ROWS, (r + 1) * ROWS)
        in_maps.append(
            {
                "lc": lc,
                "csq": csq,
                "cov9": cov9,
                "cov3": cov3,
                "ctl": np.ascontiguousarray(c[:, sl]),
                "csql": np.ascontiguousarray(csq[:, sl]),
                "pcol": pcol,
            }
        )
    return in_maps


def kernel(space_probs, cov_inv):
    from concourse.bass_utils import run_bass_kernel_spmd

    nc = _get_program()
    in_maps = _host_inputs(space_probs, cov_inv)
    res = run_bass_kernel_spmd(nc, in_maps, list(range(NCORES)))
    out = np.concatenate(
        [res.results[r]["out"].reshape(-1) for r in range(NCORES)]
    )
    return out.reshape(GRID).astype(np.float32)
